# revision 5
# baseline (speedup 1.0000x reference)
"""Depthwise-separable conv (3x3 depthwise rank-1 + 1x1 pointwise) on 8
Trainium2 NeuronCores.

Sharding: data-parallel over batch — 2 images per core. fp16 I/O (gate
admits it): per-core HBM traffic 8.4 MiB in + 16.8 MiB out ~= 70-75 us
per rep at the DMA roofline — the target wall.

Hybrid tap scheme: with all three row-conv taps folded into the PE
(3 fp16 matmuls per PSUM bank) the PE floor is ~84-87 us/rep — above
the DMA wall. For N2TAP of the 8 sub-slabs per image ("2-tap mode") one
tap is lifted out of the PE: u = b0*y1'(w-1) + y1'(w) (b0 = row_0/row_1)
is computed by an ACT per-channel scaled copy plus a GpSimd(Pool)
tensor_tensor add — the only engines with slack, and the only ones
walrus allows to do this (Pool rejects scalar_tensor_tensor, ACT cannot
add two tensors) — and the PE accumulates just 2 matmuls per bank:
W1@u + W2@y1'(w+1), reusing the same wfold weights. Per-rep engine
budgets: PE ~72, DVE ~70 (column conv only), ACT ~74 (32 evacs + 8
scaled copies), Pool ~33, DMA ~70.

Per-core algorithm:
  1. Each fp16 image is DMA'd WHOLE into one of two persistent SBUF
     tiles of 130 rows whose first/last rows are zeroed once — zero-pad
     rows come for free. Prefetch is ONE IMAGE AHEAD: _image(g) issues
     the input DMA for image g+1 (18-row head first so compute can
     start early). All input+output DMAs are issued by SP (HWDGE path),
     which carries nothing else; program order in(g+1) < out(g,...)
     keeps the prefetch from parking behind output-DMA semaphore waits.
  2. Column conv (3 taps along H, per-channel scalars) per 16-row
     sub-slab: 2 DVE scalar_tensor_tensor ops, y1' = (x_up*a0 +
     x_center) + x_down*a2 with a_i = col_i/col_1 and col_1 folded into
     the matmul weights. y1' rows use a 130-element stride whose zeroed
     2-element inter-row pad provides zero-pad edge semantics (six
     persistent y1 tiles, pads zeroed once).
  3. 3-tap sub-slabs: row conv + pointwise folded into PE — 3
     accumulated fp16 matmuls per 512-col PSUM bank, w-shifts as +j
     access-pattern offsets into padded y1'. 2-tap sub-slabs: ACT+Pool
     compute u, then 2 accumulated matmuls per bank.
  4. ACT evacuates each PSUM tile (4 banks = one sub-slab x one oc
     half) with a single N=2048 f32->fp16 copy into 32-row staging
     tiles; SP issues the output DMAs.
"""
import sys

sys.path.insert(0, "/opt/trn_rl_repo")

from contextlib import ExitStack

import numpy as np

import concourse.tile as tile
from concourse import bacc, mybir
from concourse.bass_utils import run_bass_kernel_spmd

F32 = mybir.dt.float32
F16 = mybir.dt.float16

B, C, H, W = 16, 128, 128, 128
OUT = 256
N_CORES = 8
B_LOC = B // N_CORES          # images per core
SUB = 16                      # rows per col-pass sub-slab / psum tile
N_SUB = H // SUB              # 8 sub-slabs per image
WP = W + 2                    # padded y1 row stride
N_Y1 = 6                      # persistent y1 tiles (pipeline depth)
N_U = 4                       # persistent u tiles (2-tap row-conv partial)
OROWS = 32                    # output staging granularity (rows)
N2TAP = 4                     # sub-slabs per image in 2-tap mode

LAST_EXEC_NS = None

_CACHED_NC = None


def _mode2_set(n2):
    """Which sub-slab indices run 2-tap mode — spread across the image.
    Returns {ss: route} where route alternates between 'act' (scaled
    copy on ACT) and 'pool' (tensor_scalar on Pool) so neither engine
    absorbs the whole lift load."""
    if n2 <= 0:
        return {}
    step = N_SUB / n2
    return {int((i + 0.5) * step): ("pool" if i % 2 == 0 else "act")
            for i in range(n2)}


def _build(repeat=1, factored=True, rfact=True, n2=N2TAP):
    """factored: column conv as y1' = a0*x_up + x_center + a2*x_dn
    (a_i = col_i/col_1 folded on host; col_1 absorbed into the matmul
    weights) — 2 DVE stt ops per sub-slab. factored=False: classic 3-op
    column pass (ACT center mul + 2 stt). rfact: row kernel factorable
    by row_1 (enables 2-tap mode); n2 forced to 0 when False."""
    if not rfact:
        n2 = 0
    mode2 = _mode2_set(n2)
    nc = bacc.Bacc(trn_type="TRN2", target_bir_lowering=False, debug=False)
    xin = nc.dram_tensor("xin", [B_LOC, C, H, W], F16, kind="ExternalInput").ap()
    wfold = nc.dram_tensor("wfold", [3, C, OUT], F16, kind="ExternalInput").ap()
    colk = nc.dram_tensor("colk", [C, 5], F32, kind="ExternalInput").ap()
    out = nc.dram_tensor("out", [B_LOC, OUT, H, W], F16, kind="ExternalOutput").ap()

    n_img = repeat * B_LOC

    with tile.TileContext(nc) as tc, ExitStack() as ctx:
        wpool = ctx.enter_context(tc.tile_pool(name="weights", bufs=1))
        opool = ctx.enter_context(tc.tile_pool(name="out", bufs=4))
        pspool = ctx.enter_context(tc.tile_pool(name="ps", bufs=2, space="PSUM"))

        w_t = wpool.tile([C, 3 * OUT], F16, tag="w")
        for j in range(3):
            nc.sync.dma_start(w_t[:, j * OUT:(j + 1) * OUT], wfold[j])
        ck = wpool.tile([C, 5], F32, tag="ck")
        nc.sync.dma_start(ck[:], colk[:])

        # two persistent whole-image x tiles, 130 rows: row 0 and row 129
        # are zero-pad rows memset once and never rewritten (image DMAs
        # only touch rows 1..128)
        x_ts = [wpool.tile([C, (H + 2) * W], F16, tag=f"x_{i}",
                           name=f"x_{i}") for i in range(2)]
        for x_t in x_ts:
            nc.gpsimd.memset(x_t[:, 0:W], 0.0)
            nc.gpsimd.memset(x_t[:, (H + 1) * W:], 0.0)

        # persistent y1 tiles; inter-row pad columns zeroed once
        y1s = [wpool.tile([C, (SUB + 1) * WP], F16, tag=f"y1_{i}",
                          name=f"y1_{i}")
               for i in range(N_Y1)]
        for y1 in y1s:
            nc.vector.memset(
                y1[:].rearrange("c (h w) -> c h w", w=WP)[:, :, 0:2]
                .bitcast(F32), 0.0)
        # 2-tap row-conv partial tiles (unpadded, fully written each use)
        uts = [wpool.tile([C, SUB * W], F16, tag=f"u_{i}", name=f"u_{i}")
               for i in range(N_U)] if mode2 else []

        def wj(j, oc):  # lhsT [C=128, O=128] for tap j, out-channel half oc
            return w_t[:, j * OUT + oc * 128: j * OUT + oc * 128 + 128]

        def prefetch(g):
            """Input DMA for global image g (split: 18-row head so the
            first sub-slab's column pass can begin early)."""
            b = g % B_LOC
            x_t = x_ts[g % 2]
            nc.sync.dma_start(x_t[:, W:(SUB + 2) * W], xin[b, :, 0:SUB + 1, :])
            nc.sync.dma_start(x_t[:, (SUB + 2) * W:(H + 1) * W],
                              xin[b, :, SUB + 1:H, :])

        prefetch(0)
        for g in range(n_img):
            _image(nc, tc, xin, out, x_ts[g % 2], opool, pspool, wj,
                   ck, y1s, uts, mode2, g, prefetch, factored, n_img)
    nc.compile()
    return nc


def _image(nc, tc, xin, out, x_t, opool, pspool, wj, ck, y1s, uts, mode2,
           g, prefetch, factored, n_img):
    b = g % B_LOC
    last = g == n_img - 1
    if not last:
        prefetch(g + 1)
    x3 = x_t[:].rearrange("c (h w) -> c h w", w=W)

    ot = None
    spt = OROWS // SUB           # sub-slabs per output staging tile
    for ss in range(N_SUB):
        base = ss * SUB          # tile row of the sub-slab's x_up row
        if ss % spt == 0:
            ot = [opool.tile([C, OROWS * W], F16, tag="ot",
                             name=f"ot_{g}_{ss}_{oc}")
                  for oc in range(2)]
        # y1 sub-tile: SUB data rows, row stride W+2; y1[h][w] lives at
        # offset 2 + h*WP + w; zeroed pads (offsets h*WP, h*WP+1) give the
        # row-conv taps zero-pad edge semantics: tap j of rows r0..r0+3
        # is ypj[j][:, r0:r0+4, 0:W] reading offsets 1+j + h*WP + w.
        y1 = y1s[(g * N_SUB + ss) % N_Y1]
        yp = y1[:].rearrange("c (h w) -> c h w", w=WP)
        ypj = [y1[:, 1 + j:1 + j + SUB * WP]
               .rearrange("c (h w) -> c h w", w=WP)
               for j in range(3)]
        yd = yp[:, 0:SUB, 2:WP]       # data view [C, SUB, W]
        if factored:
            # y1' = (x_up * a0) + x_center ; y1' += x_down * a2
            nc.vector.scalar_tensor_tensor(
                yd, x3[:, base:base + SUB, :], ck[:, 0:1],
                x3[:, base + 1:base + SUB + 1, :],
                op0=mybir.AluOpType.mult, op1=mybir.AluOpType.add)
            nc.vector.scalar_tensor_tensor(
                yd, x3[:, base + 2:base + SUB + 2, :], ck[:, 2:3], yd,
                op0=mybir.AluOpType.mult, op1=mybir.AluOpType.add)
        else:
            nc.scalar.activation(
                yd, x3[:, base + 1:base + SUB + 1, :],
                mybir.ActivationFunctionType.Copy, scale=ck[:, 1:2])
            nc.vector.scalar_tensor_tensor(
                yd, x3[:, base:base + SUB, :], ck[:, 0:1], yd,
                op0=mybir.AluOpType.mult, op1=mybir.AluOpType.add)
            nc.vector.scalar_tensor_tensor(
                yd, x3[:, base + 2:base + SUB + 2, :], ck[:, 2:3], yd,
                op0=mybir.AluOpType.mult, op1=mybir.AluOpType.add)

        two_tap = ss in mode2
        if two_tap:
            # lift tap 0 out of the PE: u = b0*y1'(w-1) + y1'(w) via a
            # per-channel scaled copy (on ACT or Pool per the route
            # table) then a Pool add (in-place)
            u = uts[(g * N_SUB + ss) % N_U]
            uv = u[:].rearrange("c (h w) -> c h w", w=W)
            if mode2[ss] == "act":
                nc.scalar.activation(
                    uv, ypj[0][:, 0:SUB, 0:W],
                    mybir.ActivationFunctionType.Copy, scale=ck[:, 3:4])
            else:
                nc.gpsimd.tensor_scalar_mul(
                    uv, ypj[0][:, 0:SUB, 0:W], ck[:, 3:4])
            nc.gpsimd.tensor_tensor(
                uv, uv, ypj[1][:, 0:SUB, 0:W], op=mybir.AluOpType.add)

        # row conv + pointwise folded into PE (fp16 matmuls). One PSUM
        # tile = 4 banks = the whole 16-row sub-slab for one oc half;
        # each 512-col bank is its own accumulation group (3 taps, or 2
        # in 2-tap mode: W1@u + W2@y1'(w+1)). ACT evacuates the tile
        # with a single N=2048 f32->fp16 copy.
        for oc in range(2):
            ps = pspool.tile([128, SUB * W], F32, tag="ps")
            for q in range(SUB * W // 512):
                r0 = q * 4
                if two_tap:
                    nc.tensor.matmul(
                        ps[:, q * 512:(q + 1) * 512], wj(1, oc),
                        u[:].rearrange("c (h w) -> c h w", w=W)
                        [:, r0:r0 + 4, 0:W],
                        start=True, stop=False)
                    nc.tensor.matmul(
                        ps[:, q * 512:(q + 1) * 512], wj(2, oc),
                        ypj[2][:, r0:r0 + 4, 0:W],
                        start=False, stop=True)
                else:
                    for jx in range(3):
                        nc.tensor.matmul(
                            ps[:, q * 512:(q + 1) * 512], wj(jx, oc),
                            ypj[jx][:, r0:r0 + 4, 0:W],
                            start=(jx == 0), stop=(jx == 2))
            nc.scalar.copy(
                ot[oc][:, (ss % spt) * SUB * W:(ss % spt + 1) * SUB * W],
                ps[:])

        # Output DMAs are issued by SP (HWDGE): SP carries only DMA
        # issue, and the in(g+1) < out(g,..) program order keeps the
        # prefetch ahead of the output-DMA semaphore waits.
        if last and ss >= N_SUB - spt:
            # drain faster: per-sub-slab (16-row) DMAs at the very end
            hr = ss * SUB
            for oc in range(2):
                nc.sync.dma_start(
                    out[b, oc * 128:(oc + 1) * 128, hr:hr + SUB, :],
                    ot[oc][:, (ss % spt) * SUB * W:(ss % spt + 1) * SUB * W])
        elif ss % spt == spt - 1:
            hr = (ss - spt + 1) * SUB
            for oc in range(2):
                nc.sync.dma_start(
                    out[b, oc * 128:(oc + 1) * 128, hr:hr + OROWS, :],
                    ot[oc][:])


def host_prep(col_kernel, row_kernel, pw_weight):
    """Fold weights on the host. Returns (factored, rfact,
    wfold [3,C,OUT] fp16, colk [C,5] f32)."""
    colk3 = np.asarray(col_kernel, dtype=np.float64).reshape(C, 3)
    rowk3 = np.asarray(row_kernel, dtype=np.float64).reshape(C, 3)
    pw = np.asarray(pw_weight, dtype=np.float64)

    c1 = colk3[:, 1]
    factored = bool(np.abs(c1).min() > 1e-3)
    r1 = rowk3[:, 1]
    rfact = bool(np.abs(r1).min() > 1e-3)
    # Wj[c, o] = pw[o,c] * row[c,j]  (times c1[c] when factored)
    wfold = pw.T[None, :, :] * rowk3.T[:, :, None]      # [3, C, OUT]
    if factored:
        wfold = wfold * c1[None, :, None]
        ck3 = np.stack([colk3[:, 0] / c1, c1, colk3[:, 2] / c1], axis=1)
    else:
        ck3 = colk3
    if rfact:
        ckr = np.stack([rowk3[:, 0] / r1, rowk3[:, 2] / r1], axis=1)
    else:
        ckr = np.zeros((C, 2))
    ck = np.concatenate([ck3, ckr], axis=1)             # [C, 5]
    return (factored, rfact,
            np.ascontiguousarray(wfold).astype(np.float16),
            np.ascontiguousarray(ck).astype(np.float32))


def kernel(x, col_kernel, row_kernel, pw_weight, trace=False):
    global LAST_EXEC_NS, _CACHED_NC
    x = np.ascontiguousarray(np.asarray(x).astype(np.float16))
    factored, rfact, wfold, colk = host_prep(col_kernel, row_kernel, pw_weight)

    key = (factored, rfact)
    if _CACHED_NC is None or _CACHED_NC[1] != key:
        _CACHED_NC = (_build(factored=factored, rfact=rfact), key)
    nc = _CACHED_NC[0]

    in_maps = [
        {"xin": np.ascontiguousarray(x[i * B_LOC:(i + 1) * B_LOC]),
         "wfold": wfold, "colk": colk}
        for i in range(N_CORES)
    ]
    res = run_bass_kernel_spmd(nc, in_maps, list(range(N_CORES)), trace=trace)
    LAST_EXEC_NS = res.exec_time_ns
    return np.concatenate(
        [res.results[i]["out"].astype(np.float32) for i in range(N_CORES)],
        axis=0)


# revision 8
# speedup vs baseline: 1.0815x; 1.0815x over previous
"""Depthwise-separable conv (3x3 depthwise rank-1 + 1x1 pointwise) on 8
Trainium2 NeuronCores.

Sharding: data-parallel over batch — 2 images per core. fp16 I/O (gate
admits it): per-core HBM traffic 8.4 MiB in + 16.8 MiB out ~= 70-75 us
per rep at the DMA roofline — the target wall.

Hybrid tap scheme: with all three row-conv taps folded into the PE
(3 fp16 matmuls per PSUM bank) the PE floor is ~84-87 us/rep — above
the DMA wall. For N2TAP of the 8 sub-slabs per image ("2-tap mode") one
tap is lifted out of the PE: u = b0*y1'(w-1) + y1'(w) (b0 = row_0/row_1)
is computed by an ACT per-channel scaled copy plus a GpSimd(Pool)
tensor_tensor add — the only engines with slack, and the only ones
walrus allows to do this (Pool rejects scalar_tensor_tensor, ACT cannot
add two tensors) — and the PE accumulates just 2 matmuls per bank:
W1@u + W2@y1'(w+1), reusing the same wfold weights. Per-rep engine
budgets: PE ~72, DVE ~70 (column conv only), ACT ~74 (32 evacs + 8
scaled copies), Pool ~33, DMA ~70.

Per-core algorithm:
  1. Each fp16 image is DMA'd WHOLE into one of two persistent SBUF
     tiles of 130 rows whose first/last rows are zeroed once — zero-pad
     rows come for free. Prefetch is ONE IMAGE AHEAD: _image(g) issues
     the input DMA for image g+1 (18-row head first so compute can
     start early). All input+output DMAs are issued by SP (HWDGE path),
     which carries nothing else; program order in(g+1) < out(g,...)
     keeps the prefetch from parking behind output-DMA semaphore waits.
  2. Column conv (3 taps along H, per-channel scalars) per 16-row
     sub-slab: 2 DVE scalar_tensor_tensor ops, y1' = (x_up*a0 +
     x_center) + x_down*a2 with a_i = col_i/col_1 and col_1 folded into
     the matmul weights. y1' rows use a 130-element stride whose zeroed
     2-element inter-row pad provides zero-pad edge semantics (six
     persistent y1 tiles, pads zeroed once).
  3. 3-tap sub-slabs: row conv + pointwise folded into PE — 3
     accumulated fp16 matmuls per 512-col PSUM bank, w-shifts as +j
     access-pattern offsets into padded y1'. 2-tap sub-slabs: ACT+Pool
     compute u, then 2 accumulated matmuls per bank.
  4. ACT evacuates each PSUM tile (4 banks = one sub-slab x one oc
     half) with a single N=2048 f32->fp16 copy into 32-row staging
     tiles; SP issues the output DMAs.
"""
import sys

sys.path.insert(0, "/opt/trn_rl_repo")

from contextlib import ExitStack

import numpy as np

import concourse.tile as tile
from concourse import bacc, mybir
from concourse.bass_utils import run_bass_kernel_spmd

F32 = mybir.dt.float32
F16 = mybir.dt.float16

B, C, H, W = 16, 128, 128, 128
OUT = 256
N_CORES = 8
B_LOC = B // N_CORES          # images per core
SUB = 16                      # rows per col-pass sub-slab / psum tile
N_SUB = H // SUB              # 8 sub-slabs per image
WP = W + 2                    # padded y1 row stride
N_Y1 = 6                      # persistent y1 tiles (pipeline depth)
N_U = 4                       # persistent u tiles (2-tap row-conv partial)
OROWS = 32                    # output staging granularity (rows)
N2TAP = 5                     # sub-slabs per image in 2-tap mode

LAST_EXEC_NS = None

_CACHED_NC = None


def _mode2_set(n2):
    """Which sub-slab indices run 2-tap mode — spread across the image.
    Returns {ss: route} where route alternates between 'act' (scaled
    copy on ACT) and 'pool' (tensor_scalar on Pool) so neither engine
    absorbs the whole lift load."""
    if n2 <= 0:
        return {}
    if n2 == 5:
        # 5 lifts: one on ACT (it has ~10 us slack after evacs), the
        # rest on Pool
        return {0: "pool", 1: "act", 3: "pool", 5: "pool", 7: "pool"}
    step = N_SUB / n2
    return {int((i + 0.5) * step): ("pool" if i % 2 == 0 else "act")
            for i in range(n2)}


def _build(repeat=1, factored=True, rfact=True, n2=N2TAP):
    """factored: column conv as y1' = a0*x_up + x_center + a2*x_dn
    (a_i = col_i/col_1 folded on host; col_1 absorbed into the matmul
    weights) — 2 DVE stt ops per sub-slab. factored=False: classic 3-op
    column pass (ACT center mul + 2 stt). rfact: row kernel factorable
    by row_1 (enables 2-tap mode); n2 forced to 0 when False."""
    if not rfact:
        n2 = 0
    mode2 = _mode2_set(n2)
    nc = bacc.Bacc(trn_type="TRN2", target_bir_lowering=False, debug=False)
    xin = nc.dram_tensor("xin", [B_LOC, C, H, W], F16, kind="ExternalInput").ap()
    wfold = nc.dram_tensor("wfold", [3, C, OUT], F16, kind="ExternalInput").ap()
    colk = nc.dram_tensor("colk", [C, 5], F32, kind="ExternalInput").ap()
    out = nc.dram_tensor("out", [B_LOC, OUT, H, W], F16, kind="ExternalOutput").ap()

    n_img = repeat * B_LOC

    with tile.TileContext(nc) as tc, ExitStack() as ctx:
        wpool = ctx.enter_context(tc.tile_pool(name="weights", bufs=1))
        opool = ctx.enter_context(tc.tile_pool(name="out", bufs=4))
        pspool = ctx.enter_context(tc.tile_pool(name="ps", bufs=2, space="PSUM"))

        w_t = wpool.tile([C, 3 * OUT], F16, tag="w")
        for j in range(3):
            nc.sync.dma_start(w_t[:, j * OUT:(j + 1) * OUT], wfold[j])
        ck = wpool.tile([C, 5], F32, tag="ck")
        nc.sync.dma_start(ck[:], colk[:])

        # two persistent whole-image x tiles, 130 rows: row 0 and row 129
        # are zero-pad rows memset once and never rewritten (image DMAs
        # only touch rows 1..128)
        x_ts = [wpool.tile([C, (H + 2) * W], F16, tag=f"x_{i}",
                           name=f"x_{i}") for i in range(2)]
        for x_t in x_ts:
            nc.gpsimd.memset(x_t[:, 0:W], 0.0)
            nc.gpsimd.memset(x_t[:, (H + 1) * W:], 0.0)

        # persistent y1 tiles; inter-row pad columns zeroed once
        y1s = [wpool.tile([C, (SUB + 1) * WP], F16, tag=f"y1_{i}",
                          name=f"y1_{i}")
               for i in range(N_Y1)]
        for y1 in y1s:
            nc.vector.memset(
                y1[:].rearrange("c (h w) -> c h w", w=WP)[:, :, 0:2]
                .bitcast(F32), 0.0)
        # 2-tap row-conv partial tiles (unpadded, fully written each use)
        uts = [wpool.tile([C, SUB * W], F16, tag=f"u_{i}", name=f"u_{i}")
               for i in range(N_U)] if mode2 else []

        def wj(j, oc):  # lhsT [C=128, O=128] for tap j, out-channel half oc
            return w_t[:, j * OUT + oc * 128: j * OUT + oc * 128 + 128]

        def prefetch(g):
            """Input DMA for global image g. Split: 18-row head so the
            first sub-slab's column pass can begin early, then ~37-row
            chunks — chunking keeps any single transfer from parking the
            DMA engines long enough to starve the output-DMA stream."""
            b = g % B_LOC
            x_t = x_ts[g % 2]
            nc.sync.dma_start(x_t[:, W:(SUB + 2) * W], xin[b, :, 0:SUB + 1, :])
            r = SUB + 1
            while r < H:
                r2 = min(r + 37, H)
                nc.sync.dma_start(x_t[:, (r + 1) * W:(r2 + 1) * W],
                                  xin[b, :, r:r2, :])
                r = r2

        prefetch(0)
        for g in range(n_img):
            _image(nc, tc, xin, out, x_ts[g % 2], opool, pspool, wj,
                   ck, y1s, uts, mode2, g, prefetch, factored, n_img)
    nc.compile()
    return nc


def _image(nc, tc, xin, out, x_t, opool, pspool, wj, ck, y1s, uts, mode2,
           g, prefetch, factored, n_img):
    b = g % B_LOC
    last = g == n_img - 1
    if not last:
        prefetch(g + 1)
    x3 = x_t[:].rearrange("c (h w) -> c h w", w=W)

    ot = None
    spt = OROWS // SUB           # sub-slabs per output staging tile
    for ss in range(N_SUB):
        base = ss * SUB          # tile row of the sub-slab's x_up row
        if ss % spt == 0:
            ot = [opool.tile([C, OROWS * W], F16, tag="ot",
                             name=f"ot_{g}_{ss}_{oc}")
                  for oc in range(2)]
        # y1 sub-tile: SUB data rows, row stride W+2; y1[h][w] lives at
        # offset 2 + h*WP + w; zeroed pads (offsets h*WP, h*WP+1) give the
        # row-conv taps zero-pad edge semantics: tap j of rows r0..r0+3
        # is ypj[j][:, r0:r0+4, 0:W] reading offsets 1+j + h*WP + w.
        y1 = y1s[(g * N_SUB + ss) % N_Y1]
        yp = y1[:].rearrange("c (h w) -> c h w", w=WP)
        ypj = [y1[:, 1 + j:1 + j + SUB * WP]
               .rearrange("c (h w) -> c h w", w=WP)
               for j in range(3)]
        yd = yp[:, 0:SUB, 2:WP]       # data view [C, SUB, W]
        if factored:
            # y1' = (x_up * a0) + x_center ; y1' += x_down * a2
            nc.vector.scalar_tensor_tensor(
                yd, x3[:, base:base + SUB, :], ck[:, 0:1],
                x3[:, base + 1:base + SUB + 1, :],
                op0=mybir.AluOpType.mult, op1=mybir.AluOpType.add)
            nc.vector.scalar_tensor_tensor(
                yd, x3[:, base + 2:base + SUB + 2, :], ck[:, 2:3], yd,
                op0=mybir.AluOpType.mult, op1=mybir.AluOpType.add)
        else:
            nc.scalar.activation(
                yd, x3[:, base + 1:base + SUB + 1, :],
                mybir.ActivationFunctionType.Copy, scale=ck[:, 1:2])
            nc.vector.scalar_tensor_tensor(
                yd, x3[:, base:base + SUB, :], ck[:, 0:1], yd,
                op0=mybir.AluOpType.mult, op1=mybir.AluOpType.add)
            nc.vector.scalar_tensor_tensor(
                yd, x3[:, base + 2:base + SUB + 2, :], ck[:, 2:3], yd,
                op0=mybir.AluOpType.mult, op1=mybir.AluOpType.add)

        two_tap = ss in mode2
        if two_tap:
            # lift tap 0 out of the PE: u = b0*y1'(w-1) + y1'(w) via a
            # per-channel scaled copy (on ACT or Pool per the route
            # table) then a Pool add (in-place)
            u = uts[(g * N_SUB + ss) % N_U]
            uv = u[:].rearrange("c (h w) -> c h w", w=W)
            if mode2[ss] == "act":
                nc.scalar.activation(
                    uv, ypj[0][:, 0:SUB, 0:W],
                    mybir.ActivationFunctionType.Copy, scale=ck[:, 3:4])
            else:
                nc.gpsimd.tensor_scalar_mul(
                    uv, ypj[0][:, 0:SUB, 0:W], ck[:, 3:4])
            nc.gpsimd.tensor_tensor(
                uv, uv, ypj[1][:, 0:SUB, 0:W], op=mybir.AluOpType.add)

        # row conv + pointwise folded into PE (fp16 matmuls). One PSUM
        # tile = 4 banks = the whole 16-row sub-slab for one oc half;
        # each 512-col bank is its own accumulation group (3 taps, or 2
        # in 2-tap mode: W1@u + W2@y1'(w+1)). ACT evacuates the tile
        # with a single N=2048 f32->fp16 copy.
        for oc in range(2):
            ps = pspool.tile([128, SUB * W], F32, tag="ps")
            for q in range(SUB * W // 512):
                r0 = q * 4
                if two_tap:
                    nc.tensor.matmul(
                        ps[:, q * 512:(q + 1) * 512], wj(1, oc),
                        u[:].rearrange("c (h w) -> c h w", w=W)
                        [:, r0:r0 + 4, 0:W],
                        start=True, stop=False)
                    nc.tensor.matmul(
                        ps[:, q * 512:(q + 1) * 512], wj(2, oc),
                        ypj[2][:, r0:r0 + 4, 0:W],
                        start=False, stop=True)
                else:
                    for jx in range(3):
                        nc.tensor.matmul(
                            ps[:, q * 512:(q + 1) * 512], wj(jx, oc),
                            ypj[jx][:, r0:r0 + 4, 0:W],
                            start=(jx == 0), stop=(jx == 2))
            nc.scalar.copy(
                ot[oc][:, (ss % spt) * SUB * W:(ss % spt + 1) * SUB * W],
                ps[:])

        # Output DMAs are issued by SP (HWDGE): SP carries only DMA
        # issue, and the in(g+1) < out(g,..) program order keeps the
        # prefetch ahead of the output-DMA semaphore waits.
        if last and ss >= N_SUB - spt:
            # drain faster: per-sub-slab (16-row) DMAs at the very end
            hr = ss * SUB
            for oc in range(2):
                nc.sync.dma_start(
                    out[b, oc * 128:(oc + 1) * 128, hr:hr + SUB, :],
                    ot[oc][:, (ss % spt) * SUB * W:(ss % spt + 1) * SUB * W])
        elif ss % spt == spt - 1:
            hr = (ss - spt + 1) * SUB
            for oc in range(2):
                nc.sync.dma_start(
                    out[b, oc * 128:(oc + 1) * 128, hr:hr + OROWS, :],
                    ot[oc][:])


def host_prep(col_kernel, row_kernel, pw_weight):
    """Fold weights on the host. Returns (factored, rfact,
    wfold [3,C,OUT] fp16, colk [C,5] f32)."""
    colk3 = np.asarray(col_kernel, dtype=np.float64).reshape(C, 3)
    rowk3 = np.asarray(row_kernel, dtype=np.float64).reshape(C, 3)
    pw = np.asarray(pw_weight, dtype=np.float64)

    c1 = colk3[:, 1]
    factored = bool(np.abs(c1).min() > 1e-3)
    r1 = rowk3[:, 1]
    rfact = bool(np.abs(r1).min() > 1e-3)
    # Wj[c, o] = pw[o,c] * row[c,j]  (times c1[c] when factored)
    wfold = pw.T[None, :, :] * rowk3.T[:, :, None]      # [3, C, OUT]
    if factored:
        wfold = wfold * c1[None, :, None]
        ck3 = np.stack([colk3[:, 0] / c1, c1, colk3[:, 2] / c1], axis=1)
    else:
        ck3 = colk3
    if rfact:
        ckr = np.stack([rowk3[:, 0] / r1, rowk3[:, 2] / r1], axis=1)
    else:
        ckr = np.zeros((C, 2))
    ck = np.concatenate([ck3, ckr], axis=1)             # [C, 5]
    return (factored, rfact,
            np.ascontiguousarray(wfold).astype(np.float16),
            np.ascontiguousarray(ck).astype(np.float32))


def kernel(x, col_kernel, row_kernel, pw_weight, trace=False):
    global LAST_EXEC_NS, _CACHED_NC
    x = np.ascontiguousarray(np.asarray(x).astype(np.float16))
    factored, rfact, wfold, colk = host_prep(col_kernel, row_kernel, pw_weight)

    key = (factored, rfact)
    if _CACHED_NC is None or _CACHED_NC[1] != key:
        _CACHED_NC = (_build(factored=factored, rfact=rfact), key)
    nc = _CACHED_NC[0]

    in_maps = [
        {"xin": np.ascontiguousarray(x[i * B_LOC:(i + 1) * B_LOC]),
         "wfold": wfold, "colk": colk}
        for i in range(N_CORES)
    ]
    res = run_bass_kernel_spmd(nc, in_maps, list(range(N_CORES)), trace=trace)
    LAST_EXEC_NS = res.exec_time_ns
    return np.concatenate(
        [res.results[i]["out"].astype(np.float32) for i in range(N_CORES)],
        axis=0)


# revision 11
# speedup vs baseline: 3.3527x; 3.1001x over previous
"""Depthwise-separable conv (3x3 depthwise rank-1 + 1x1 pointwise) on 8
Trainium2 NeuronCores.

Sharding: data-parallel over batch — 2 images per core. fp16 I/O (gate
admits it): per-core HBM traffic 8.4 MiB in + 16.8 MiB out ~= 70-75 us
per rep at the DMA roofline — the target wall.

Hybrid tap scheme: with all three row-conv taps folded into the PE
(3 fp16 matmuls per PSUM bank) the PE floor is ~84-87 us/rep — above
the DMA wall. For N2TAP of the 8 sub-slabs per image ("2-tap mode") one
tap is lifted out of the PE: u = b0*y1'(w-1) + y1'(w) (b0 = row_0/row_1)
is computed by an ACT per-channel scaled copy plus a GpSimd(Pool)
tensor_tensor add — the only engines with slack, and the only ones
walrus allows to do this (Pool rejects scalar_tensor_tensor, ACT cannot
add two tensors) — and the PE accumulates just 2 matmuls per bank:
W1@u + W2@y1'(w+1), reusing the same wfold weights. Per-rep engine
budgets: PE ~72, DVE ~70 (column conv only), ACT ~74 (32 evacs + 8
scaled copies), Pool ~33, DMA ~70.

Per-core algorithm:
  1. Each fp16 image is DMA'd WHOLE into one of two persistent SBUF
     tiles of 130 rows whose first/last rows are zeroed once — zero-pad
     rows come for free. Prefetch is ONE IMAGE AHEAD: _image(g) issues
     the input DMA for image g+1 (18-row head first so compute can
     start early). All input+output DMAs are issued by SP (HWDGE path),
     which carries nothing else; program order in(g+1) < out(g,...)
     keeps the prefetch from parking behind output-DMA semaphore waits.
  2. Column conv (3 taps along H, per-channel scalars) per 16-row
     sub-slab: 2 DVE scalar_tensor_tensor ops, y1' = (x_up*a0 +
     x_center) + x_down*a2 with a_i = col_i/col_1 and col_1 folded into
     the matmul weights. y1' rows use a 130-element stride whose zeroed
     2-element inter-row pad provides zero-pad edge semantics (six
     persistent y1 tiles, pads zeroed once).
  3. 3-tap sub-slabs: row conv + pointwise folded into PE — 3
     accumulated fp16 matmuls per 512-col PSUM bank, w-shifts as +j
     access-pattern offsets into padded y1'. 2-tap sub-slabs: ACT+Pool
     compute u, then 2 accumulated matmuls per bank.
  4. ACT evacuates each PSUM tile (4 banks = one sub-slab x one oc
     half) with a single N=2048 f32->fp16 copy into 32-row staging
     tiles; SP issues the output DMAs.
"""
import sys

sys.path.insert(0, "/opt/trn_rl_repo")

from contextlib import ExitStack

import numpy as np

import concourse.tile as tile
from concourse import bacc, mybir
from concourse.bass_utils import run_bass_kernel_spmd

F32 = mybir.dt.float32
F16 = mybir.dt.float16

B, C, H, W = 16, 128, 128, 128
OUT = 256
N_CORES = 8
B_LOC = B // N_CORES          # images per core
SUB = 16                      # rows per col-pass sub-slab / psum tile
N_SUB = H // SUB              # 8 sub-slabs per image
WP = W + 2                    # padded y1 row stride
N_Y1 = 6                      # persistent y1 tiles (pipeline depth)
N_U = 4                       # persistent u tiles (2-tap row-conv partial)
OROWS = 32                    # output staging granularity (rows)
# per-image schedules: LIFT maps 2-tap sub-slabs to the engine route for
# the lift's scaled-copy ('pool' = Pool tt-mult with a broadcast tile,
# 'act' = ACT scaled copy); BCAST_COL sub-slabs run the column conv as
# Pool tt-mult + DVE tt-add pairs instead of 2 DVE stt ops.
LIFT = {0: "pool", 2: "pool", 3: "act", 5: "pool", 7: "pool"}
BCAST_COL = {6}
SWP = SUB * WP                # u tile length (2080)

LAST_EXEC_NS = None

_CACHED_NC = None


def _build(repeat=1, factored=True, rfact=True):
    """factored: column conv as y1' = a0*x_up + x_center + a2*x_dn
    (a_i = col_i/col_1 folded on host; col_1 absorbed into the matmul
    weights). factored=False: classic 3-op column pass (ACT center mul
    + 2 stt) and no broadcast col passes. rfact: row kernel factorable
    by row_1 (enables 2-tap lifts); lifts disabled when False."""
    mode2 = dict(LIFT) if rfact else {}
    bcast = set(BCAST_COL) if factored else set()
    nc = bacc.Bacc(trn_type="TRN2", target_bir_lowering=False, debug=False)
    xin = nc.dram_tensor("xin", [B_LOC, C, H, W], F16, kind="ExternalInput").ap()
    wfold = nc.dram_tensor("wfold", [3, C, OUT], F16, kind="ExternalInput").ap()
    colk = nc.dram_tensor("colk", [C, 5], F32, kind="ExternalInput").ap()
    out = nc.dram_tensor("out", [B_LOC, OUT, H, W], F16, kind="ExternalOutput").ap()

    n_img = repeat * B_LOC

    with tile.TileContext(nc) as tc, ExitStack() as ctx:
        wpool = ctx.enter_context(tc.tile_pool(name="weights", bufs=1))
        opool = ctx.enter_context(tc.tile_pool(name="out", bufs=4))
        pspool = ctx.enter_context(tc.tile_pool(name="ps", bufs=2, space="PSUM"))

        w_t = wpool.tile([C, 3 * OUT], F16, tag="w")
        for j in range(3):
            nc.sync.dma_start(w_t[:, j * OUT:(j + 1) * OUT], wfold[j])
        ck = wpool.tile([C, 5], F32, tag="ck")
        nc.sync.dma_start(ck[:], colk[:])

        # two persistent whole-image x tiles, 130 rows: row 0 and row 129
        # are zero-pad rows memset once and never rewritten (image DMAs
        # only touch rows 1..128)
        x_ts = [wpool.tile([C, (H + 2) * W], F16, tag=f"x_{i}",
                           name=f"x_{i}") for i in range(2)]
        for x_t in x_ts:
            nc.gpsimd.memset(x_t[:, 0:W], 0.0)
            nc.gpsimd.memset(x_t[:, (H + 1) * W:], 0.0)

        # persistent y1 tiles; inter-row pad columns zeroed once
        y1s = [wpool.tile([C, (SUB + 1) * WP], F16, tag=f"y1_{i}",
                          name=f"y1_{i}")
               for i in range(N_Y1)]
        for y1 in y1s:
            nc.vector.memset(
                y1[:].rearrange("c (h w) -> c h w", w=WP)[:, :, 0:2]
                .bitcast(F32), 0.0)
        # 2-tap row-conv partial tiles (WP-strided layout like y1)
        uts = [wpool.tile([C, SWP], F16, tag=f"u_{i}", name=f"u_{i}")
               for i in range(N_U)] if mode2 else []
        # broadcast per-channel coefficient tiles (coef replicated along
        # the free dim) so Pool can scale via tensor_tensor MULT — its
        # tensor_scalar runs ~10x below roofline on HW. Built once at
        # startup from a ones tile via ACT scaled copies.
        t_sc = [wpool.tile([C, SUB * W], F16, tag=f"t_{i}", name=f"t_{i}")
                for i in range(2)] if bcast else []
        bc_b0 = bc_a0 = bc_a2 = None
        if mode2 or bcast:
            ones = wpool.tile([C, SWP], F16, tag="ones")
            nc.vector.memset(ones[:], 1.0)
            if mode2:
                bc_b0 = wpool.tile([C, SWP], F16, tag="bc_b0")
                nc.scalar.activation(bc_b0[:], ones[:],
                                     mybir.ActivationFunctionType.Copy,
                                     scale=ck[:, 3:4])
            if bcast:
                bc_a0 = wpool.tile([C, SUB * W], F16, tag="bc_a0")
                bc_a2 = wpool.tile([C, SUB * W], F16, tag="bc_a2")
                nc.scalar.activation(bc_a0[:], ones[:, 0:SUB * W],
                                     mybir.ActivationFunctionType.Copy,
                                     scale=ck[:, 0:1])
                nc.scalar.activation(bc_a2[:], ones[:, 0:SUB * W],
                                     mybir.ActivationFunctionType.Copy,
                                     scale=ck[:, 2:3])
        bc = (bc_b0, bc_a0, bc_a2, t_sc)

        def wj(j, oc):  # lhsT [C=128, O=128] for tap j, out-channel half oc
            return w_t[:, j * OUT + oc * 128: j * OUT + oc * 128 + 128]

        def prefetch(g):
            """Input DMA for global image g. Split: 18-row head so the
            first sub-slab's column pass can begin early, then ~37-row
            chunks — chunking keeps any single transfer from parking the
            DMA engines long enough to starve the output-DMA stream."""
            b = g % B_LOC
            x_t = x_ts[g % 2]
            nc.sync.dma_start(x_t[:, W:(SUB + 2) * W], xin[b, :, 0:SUB + 1, :])
            r = SUB + 1
            while r < H:
                r2 = min(r + 37, H)
                nc.sync.dma_start(x_t[:, (r + 1) * W:(r2 + 1) * W],
                                  xin[b, :, r:r2, :])
                r = r2

        prefetch(0)
        for g in range(n_img):
            _image(nc, tc, xin, out, x_ts[g % 2], opool, pspool, wj,
                   ck, y1s, uts, mode2, bcast, bc, g, prefetch, factored,
                   n_img)
    nc.compile()
    return nc


def _image(nc, tc, xin, out, x_t, opool, pspool, wj, ck, y1s, uts, mode2,
           bcast, bc, g, prefetch, factored, n_img):
    bc_b0, bc_a0, bc_a2, t_sc = bc
    b = g % B_LOC
    last = g == n_img - 1
    if not last:
        prefetch(g + 1)
    x3 = x_t[:].rearrange("c (h w) -> c h w", w=W)

    ot = None
    spt = OROWS // SUB           # sub-slabs per output staging tile
    for ss in range(N_SUB):
        base = ss * SUB          # tile row of the sub-slab's x_up row
        if ss % spt == 0:
            ot = [opool.tile([C, OROWS * W], F16, tag="ot",
                             name=f"ot_{g}_{ss}_{oc}")
                  for oc in range(2)]
        # y1 sub-tile: SUB data rows, row stride W+2; y1[h][w] lives at
        # offset 2 + h*WP + w; zeroed pads (offsets h*WP, h*WP+1) give the
        # row-conv taps zero-pad edge semantics: tap j of rows r0..r0+3
        # is ypj[j][:, r0:r0+4, 0:W] reading offsets 1+j + h*WP + w.
        y1 = y1s[(g * N_SUB + ss) % N_Y1]
        yp = y1[:].rearrange("c (h w) -> c h w", w=WP)
        ypj = [y1[:, 1 + j:1 + j + SUB * WP]
               .rearrange("c (h w) -> c h w", w=WP)
               for j in range(3)]
        yd = yp[:, 0:SUB, 2:WP]       # data view [C, SUB, W]
        if factored and ss in bcast:
            # broadcast-tile column pass: t_i = a_i (*) x (Pool tt MULT
            # with the coefficient broadcast tile, flat contiguous),
            # accumulated into yd by DVE tt adds (packed ~0.78 us)
            xu = x_t[:, base * W:(base + SUB) * W]
            xd = x_t[:, (base + 2) * W:(base + SUB + 2) * W]
            nc.gpsimd.tensor_tensor(t_sc[0][:], xu, bc_a0[:],
                                    op=mybir.AluOpType.mult)
            nc.vector.tensor_tensor(
                yd, t_sc[0][:].rearrange("c (h w) -> c h w", w=W),
                x3[:, base + 1:base + SUB + 1, :], op=mybir.AluOpType.add)
            nc.gpsimd.tensor_tensor(t_sc[1][:], xd, bc_a2[:],
                                    op=mybir.AluOpType.mult)
            nc.vector.tensor_tensor(
                yd, t_sc[1][:].rearrange("c (h w) -> c h w", w=W), yd,
                op=mybir.AluOpType.add)
        elif factored:
            # y1' = (x_up * a0) + x_center ; y1' += x_down * a2
            nc.vector.scalar_tensor_tensor(
                yd, x3[:, base:base + SUB, :], ck[:, 0:1],
                x3[:, base + 1:base + SUB + 1, :],
                op0=mybir.AluOpType.mult, op1=mybir.AluOpType.add)
            nc.vector.scalar_tensor_tensor(
                yd, x3[:, base + 2:base + SUB + 2, :], ck[:, 2:3], yd,
                op0=mybir.AluOpType.mult, op1=mybir.AluOpType.add)
        else:
            nc.scalar.activation(
                yd, x3[:, base + 1:base + SUB + 1, :],
                mybir.ActivationFunctionType.Copy, scale=ck[:, 1:2])
            nc.vector.scalar_tensor_tensor(
                yd, x3[:, base:base + SUB, :], ck[:, 0:1], yd,
                op0=mybir.AluOpType.mult, op1=mybir.AluOpType.add)
            nc.vector.scalar_tensor_tensor(
                yd, x3[:, base + 2:base + SUB + 2, :], ck[:, 2:3], yd,
                op0=mybir.AluOpType.mult, op1=mybir.AluOpType.add)

        two_tap = ss in mode2
        if two_tap:
            # lift tap 0 out of the PE: u[n] = b0*y1f[n+1] + y1f[n+2]
            # (flat over the WP-strided layout, n = h*WP + w; the zeroed
            # pads give w=0 edge semantics). Scaled copy on Pool (tt
            # MULT with broadcast b0 tile) or ACT per the route table;
            # the add is a packed DVE tt.
            u = uts[(g * N_SUB + ss) % N_U]
            if mode2[ss] == "act":
                nc.scalar.activation(
                    u[:], y1[:, 1:1 + SWP],
                    mybir.ActivationFunctionType.Copy, scale=ck[:, 3:4])
            else:
                nc.gpsimd.tensor_tensor(
                    u[:], y1[:, 1:1 + SWP], bc_b0[:],
                    op=mybir.AluOpType.mult)
            nc.vector.tensor_tensor(
                u[:], u[:], y1[:, 2:2 + SWP], op=mybir.AluOpType.add)

        # row conv + pointwise folded into PE (fp16 matmuls). One PSUM
        # tile = 4 banks = the whole 16-row sub-slab for one oc half;
        # each 512-col bank is its own accumulation group (3 taps, or 2
        # in 2-tap mode: W1@u + W2@y1'(w+1)). ACT evacuates the tile
        # with a single N=2048 f32->fp16 copy.
        for oc in range(2):
            ps = pspool.tile([128, SUB * W], F32, tag="ps")
            for q in range(SUB * W // 512):
                r0 = q * 4
                if two_tap:
                    nc.tensor.matmul(
                        ps[:, q * 512:(q + 1) * 512], wj(1, oc),
                        u[:].rearrange("c (h w) -> c h w", w=WP)
                        [:, r0:r0 + 4, 0:W],
                        start=True, stop=False)
                    nc.tensor.matmul(
                        ps[:, q * 512:(q + 1) * 512], wj(2, oc),
                        ypj[2][:, r0:r0 + 4, 0:W],
                        start=False, stop=True)
                else:
                    for jx in range(3):
                        nc.tensor.matmul(
                            ps[:, q * 512:(q + 1) * 512], wj(jx, oc),
                            ypj[jx][:, r0:r0 + 4, 0:W],
                            start=(jx == 0), stop=(jx == 2))
            nc.scalar.copy(
                ot[oc][:, (ss % spt) * SUB * W:(ss % spt + 1) * SUB * W],
                ps[:])

        # Output DMAs are issued by SP (HWDGE): SP carries only DMA
        # issue, and the in(g+1) < out(g,..) program order keeps the
        # prefetch ahead of the output-DMA semaphore waits.
        if last and ss >= N_SUB - spt:
            # drain faster: per-sub-slab (16-row) DMAs at the very end
            hr = ss * SUB
            for oc in range(2):
                nc.sync.dma_start(
                    out[b, oc * 128:(oc + 1) * 128, hr:hr + SUB, :],
                    ot[oc][:, (ss % spt) * SUB * W:(ss % spt + 1) * SUB * W])
        elif ss % spt == spt - 1:
            hr = (ss - spt + 1) * SUB
            for oc in range(2):
                nc.sync.dma_start(
                    out[b, oc * 128:(oc + 1) * 128, hr:hr + OROWS, :],
                    ot[oc][:])


def host_prep(col_kernel, row_kernel, pw_weight):
    """Fold weights on the host. Returns (factored, rfact,
    wfold [3,C,OUT] fp16, colk [C,5] f32)."""
    colk3 = np.asarray(col_kernel, dtype=np.float64).reshape(C, 3)
    rowk3 = np.asarray(row_kernel, dtype=np.float64).reshape(C, 3)
    pw = np.asarray(pw_weight, dtype=np.float64)

    c1 = colk3[:, 1]
    with np.errstate(divide="ignore", invalid="ignore"):
        cr = np.abs(colk3[:, [0, 2]] / c1[:, None])
        rr = np.abs(rowk3[:, [0, 2]] / rowk3[:, 1][:, None])
    factored = bool(np.abs(c1).min() > 1e-3 and cr.max() < 256)
    r1 = rowk3[:, 1]
    rfact = bool(np.abs(r1).min() > 1e-3 and rr.max() < 256)
    # Wj[c, o] = pw[o,c] * row[c,j]  (times c1[c] when factored)
    wfold = pw.T[None, :, :] * rowk3.T[:, :, None]      # [3, C, OUT]
    if factored:
        wfold = wfold * c1[None, :, None]
        ck3 = np.stack([colk3[:, 0] / c1, c1, colk3[:, 2] / c1], axis=1)
    else:
        ck3 = colk3
    if rfact:
        ckr = np.stack([rowk3[:, 0] / r1, rowk3[:, 2] / r1], axis=1)
    else:
        ckr = np.zeros((C, 2))
    ck = np.concatenate([ck3, ckr], axis=1)             # [C, 5]
    return (factored, rfact,
            np.ascontiguousarray(wfold).astype(np.float16),
            np.ascontiguousarray(ck).astype(np.float32))


def kernel(x, col_kernel, row_kernel, pw_weight, trace=False):
    global LAST_EXEC_NS, _CACHED_NC
    x = np.ascontiguousarray(np.asarray(x).astype(np.float16))
    factored, rfact, wfold, colk = host_prep(col_kernel, row_kernel, pw_weight)

    key = (factored, rfact)
    if _CACHED_NC is None or _CACHED_NC[1] != key:
        _CACHED_NC = (_build(factored=factored, rfact=rfact), key)
    nc = _CACHED_NC[0]

    in_maps = [
        {"xin": np.ascontiguousarray(x[i * B_LOC:(i + 1) * B_LOC]),
         "wfold": wfold, "colk": colk}
        for i in range(N_CORES)
    ]
    res = run_bass_kernel_spmd(nc, in_maps, list(range(N_CORES)), trace=trace)
    LAST_EXEC_NS = res.exec_time_ns
    return np.concatenate(
        [res.results[i]["out"].astype(np.float32) for i in range(N_CORES)],
        axis=0)
